# revision 5
# baseline (speedup 1.0000x reference)
"""Trainium2 Bass kernel for nn_BandSplit (banded matmul, fp8 x, chain slices).

The reference pipeline (gather -> mask -> per-band linear -> linear -> mask ->
scatter_add -> OLA) is linear in x and collapses to ONE banded matrix multiply
in the interleaved linear space lin = f*4 + c:

    out_lin[l', r] = sum_l A[l, l'] * x_lin[l, r]        (r = b*T + t rows)

A is built on the host from the (small) weight inputs.  Each core owns 4
ADJACENT 128-wide output tiles (core c -> tiles 4c..4c+3); their band supports
overlap strongly, so one shared "dense chain" of NSL=6 x-slices (128 rows
each, host-chosen offsets, consecutive gaps <= 128) covers all four supports:
tile t contracts over chain slices (t, t+1, t+2) with duplicate/out-of-band
rows zeroed in the weights.  That is 6 x-slice loads per core (vs 8 in the
per-band-slot layout) at the cost of a uniform NDP=(3,3,3,3)=12 weight blocks.

Dtypes: x quantized host-side to fp8 E3M4 (scale SX folded into A), weights
fp16, PSUM fp32, ALL outputs stored fp8 E3M4 (x SO, divided out on the host).
Bias image and the 4 output lins above 4096 (f-bin 1024) are host-side.

Per-core steady-state budget: PE 12 block-streams x 2048 cols = 24.6K cycles
~ 10.2us at 2.4GHz; DMA 3.01 MB (x 1.57 + w 0.39 + out 1.05).  Loads ride the
SP HWDGE queue, fp8 stores split across the ACT HWDGE and gpsimd SWDGE queues,
PSUM->SBUF copies on the DVE, so consecutive bodies pipeline with no
in-order-queue coupling.
"""

import numpy as np
import ml_dtypes

# ---- problem constants (hardcoded; harness supplies matching inputs) ----
B, C, T, F = 4, 4, 512, 1025
KB, WMAX = 256, 33
L = F * C                 # 4100 linear positions
R = B * T                 # 2048 rows (b, t)
NT_DEV = 32               # device out tiles (lin 0..4096); rest host residual
RES_LO = NT_DEV * 128     # 4096
NCORES = 8
CHUNK = 512               # PSUM bank (fp32) free-dim limit
NCH = R // CHUNK          # 4

# dense-chain slot structure: NSL slices, tile t reads slices (t, t+1, t+2)
NSL = 6                               # x slices per core (128 rows each)
NTPC = 4                              # out tiles per core
NDP = (3, 3, 3, 3)                    # weight blocks per tile
SMAP = tuple(tuple(range(t, t + 3)) for t in range(NTPC))
NBLK = sum(NDP)                       # 12 weight blocks per core

CORE_TILES = [tuple(range(4 * c, 4 * c + 4)) for c in range(NCORES)]

SX_TARGET = 14.8          # fp8 e3m4 max normal is 15.5; leave clip margin
SO = 5.0                  # fp8 out scale (out absmax ~2.41 -> 12.1 < 15.5)

F8 = ml_dtypes.float8_e3m4

_prog_cache = {}


def _build_program(loop_iters=1, unroll=4):
    """loop_iters counts BODY executions; the hardware loop runs
    loop_iters/unroll iterations of `unroll` pipelined bodies (the revolving
    bufs=2 pools overlap consecutive bodies; the all-engine barrier sits on
    the loop back-edge only)."""
    import concourse.bacc as bacc
    import concourse.tile as tile
    import concourse.mybir as mybir

    if unroll and loop_iters % unroll:
        unroll = 1
    key = (loop_iters, unroll)
    if key in _prog_cache:
        return _prog_cache[key]

    f32 = mybir.dt.float32
    f16 = mybir.dt.float16
    f8 = mybir.dt.float8e3

    nc = bacc.Bacc("TRN2", target_bir_lowering=False, debug=False,
                   num_devices=NCORES)
    xin = nc.dram_tensor("xin", [128, NSL * R], f8, kind="ExternalInput").ap()
    wts = nc.dram_tensor("wts", [128, NBLK * 128], f16,
                         kind="ExternalInput").ap()
    out8 = nc.dram_tensor("out8", [NTPC * 128, R], f8,
                          kind="ExternalOutput").ap()

    with tile.TileContext(nc) as tc:
        with (
            tc.tile_pool(name="xp", bufs=2) as xp,
            tc.tile_pool(name="wp", bufs=2) as wp,
            tc.tile_pool(name="y8p", bufs=2) as y8p,
            tc.tile_pool(name="pp", bufs=8, space="PSUM") as pp,
        ):
            # x DRAM layout is (chunk, slice)-interleaved: col block
            # (ch*NSL + s)*CHUNK holds chunk ch of slice s, so each chunk is
            # one contiguous 384 KB load descriptor and compute can start
            # after w0 + chunk 0.  Matmuls run chunk-major so each chunk's
            # compute chases its load.
            def body(_iv=None):
                xt = xp.tile([128, NSL * R], f8, tag="x")
                wt0 = wp.tile([128, NDP[0] * 128], f16, tag="w0")
                wtr = wp.tile([128, (NBLK - NDP[0]) * 128], f16, tag="wr")
                xo = [NSL * CHUNK * ch for ch in range(NCH + 1)]
                nc.sync.dma_start(wt0[:], wts[:, :NDP[0] * 128])
                nc.sync.dma_start(xt[:, 0:xo[1]], xin[:, 0:xo[1]])
                nc.sync.dma_start(wtr[:], wts[:, NDP[0] * 128:])
                for ch in range(1, NCH):
                    nc.sync.dma_start(xt[:, xo[ch]:xo[ch + 1]],
                                      xin[:, xo[ch]:xo[ch + 1]])

                def wblk(t, b):
                    if t == 0:
                        return wt0[:, b * 128:(b + 1) * 128]
                    blk = (sum(NDP[:t]) - NDP[0] + b) * 128
                    return wtr[:, blk:blk + 128]

                y8s = [y8p.tile([128, R], f8, tag=f"y8_{t}",
                                name=f"y8_{t}") for t in range(NTPC)]

                for ch in range(NCH):
                    order = range(NTPC) if ch == 0 else (3, 0, 1, 2)
                    for t in order:
                        ps = pp.tile([128, CHUNK], f32, tag="ps")
                        for b in range(NDP[t]):
                            c0 = xo[ch] + SMAP[t][b] * CHUNK
                            nc.tensor.matmul(
                                ps[:], wblk(t, b), xt[:, c0:c0 + CHUNK],
                                start=(b == 0), stop=(b == NDP[t] - 1),
                            )
                        # PSUM->SBUF copies (with the SO scale folded in)
                        # split across DVE (tiles 0-1) and ACT (tiles 2-3):
                        # 16 copies on one engine would BE the bottleneck
                        # (~660ns each).  Stores: tiles 0-1 ride the gpsimd
                        # SWDGE queue (DVE cannot trigger HWDGE), tiles 2-3
                        # the ACT HWDGE queue right after their own copies;
                        # both are separate from the SP load queue so
                        # consecutive bodies overlap.
                        dst = y8s[t][:, ch * CHUNK:(ch + 1) * CHUNK]
                        if t < 2:
                            nc.vector.tensor_scalar_mul(dst, ps[:], SO)
                        else:
                            nc.scalar.mul(dst, ps[:], SO)
                        if ch == NCH - 1:
                            eng = nc.gpsimd if t < 2 else nc.scalar
                            eng.dma_start(
                                out8[t * 128:(t + 1) * 128, :], y8s[t][:])

            if loop_iters == 1:
                body()
            elif unroll == 0:
                # straight-line replay (no For_i): TimelineSim cannot follow
                # reg-mode branches, so simulation uses this variant
                for _u in range(loop_iters):
                    body()
            else:
                with tc.For_i(0, loop_iters // unroll, 1) as _i:
                    for _u in range(unroll):
                        body(_i)

    nc.compile()
    _prog_cache[key] = nc
    return nc


def _build_A(pre_weight, pre_bias, post_weight, post_bias, mask, ola_window,
             f_idxes):
    """Host: banded operator A[in_lin, out_lin] and the bias image (C, F)."""
    fi = f_idxes.reshape(KB, WMAX).astype(np.int64)
    mk = mask.reshape(KB, WMAX).astype(np.float32)
    ola = ola_window.astype(np.float32)

    mrow = np.repeat(mk, C, axis=1)                     # (KB, WMAX*C)
    inv_ola = np.where(ola != 0, 1.0 / ola, 0.0)
    ola_cols = inv_ola[fi]                              # (KB, WMAX)
    mcol = np.repeat(mk * ola_cols, C, axis=1)          # (KB, WMAX*C)

    w1 = pre_weight * mrow[:, :, None]                  # (KB, D, 128)
    w2 = post_weight * mcol[:, None, :]                 # (KB, 128, D)
    Mk = np.matmul(w1, w2)                              # (KB, D, D) fp32

    LPAD = ((L + 127) // 128) * 128
    A = np.zeros((LPAD, LPAD), np.float32)
    lin = (fi[:, :, None] * C + np.arange(C)[None, None, :]).reshape(KB, -1)
    for k in range(KB):
        idx = lin[k]
        A[np.ix_(idx, idx)] += Mk[k]

    by = (np.einsum('ko,koj->kj', pre_bias, post_weight) + post_bias)
    by = by * mcol
    bias_img = np.zeros((C, F), np.float32)
    np.add.at(bias_img,
              (np.tile(np.arange(C), (KB, WMAX, 1)).reshape(KB, -1),
               np.repeat(fi, C, axis=1)),
              by)
    return A, bias_img


def _plan_slices(A):
    """Per-core chain slice offsets + per-block (tile, offset, new-row mask).

    Core c owns tiles 4c..4c+3; choose NSL non-decreasing offsets with
    consecutive gaps <= 128 such that tile i's support is inside
    [offs[i], offs[i+2]+128).  Greedy-latest: offs[i] = min(lo_i, prev+128).
    """
    sup = []
    nzc = A[:L, :RES_LO] != 0
    for j in range(NT_DEV):
        rows = np.nonzero(nzc[:, 128 * j:128 * (j + 1)].any(axis=1))[0]
        sup.append((int(rows.min()), int(rows.max())))

    slice_offs, blocks = [], []
    for c in range(NCORES):
        tiles = CORE_TILES[c]
        los = [sup[j][0] for j in tiles]
        his = [sup[j][1] for j in tiles]
        offs = []
        for i in range(NSL):
            o = L - 128
            if i < NTPC:
                o = min(o, los[i])
            if i > 0:
                o = min(o, offs[i - 1] + 128)
            offs.append(max(0, o))
        for i in range(NTPC):
            assert offs[i] <= los[i] and offs[i + 2] + 128 > his[i], \
                (c, i, offs, los, his)

        blks = []
        for t, j in enumerate(tiles):
            covered = np.zeros(L + 128, bool)
            for b in range(NDP[t]):
                o = offs[SMAP[t][b]]
                new = ~covered[o:o + 128]
                blks.append((j, o, new.copy()))
                covered[o:o + 128] = True
            assert covered[sup[j][0]:sup[j][1] + 1].all()
        slice_offs.append(offs)
        blocks.append(blks)
    return slice_offs, blocks


def _shard_inputs(x, A):
    """Per-core in_maps plus host-side residual rows (lin 4096..4099)."""
    X = np.ascontiguousarray(
        np.asarray(x, np.float32).transpose(3, 1, 0, 2).reshape(L, R))
    sx = SX_TARGET / max(float(np.abs(X).max()), 1e-30)
    Xq = np.clip(X * sx, -15.5, 15.5).astype(F8)

    slice_offs, blocks = _plan_slices(A)
    in_maps = []
    for c in range(NCORES):
        # (chunk, slice)-interleaved column layout matching _build_program
        xin = np.zeros((128, NSL * R), F8)
        for s, o in enumerate(slice_offs[c]):
            for ch in range(NCH):
                d0 = (ch * NSL + s) * CHUNK
                xin[:, d0:d0 + CHUNK] = Xq[o:o + 128,
                                           ch * CHUNK:(ch + 1) * CHUNK]
        wts = np.zeros((128, NBLK * 128), np.float32)
        for bi, (j, o, new) in enumerate(blocks[c]):
            wblk = A[o:o + 128, j * 128:(j + 1) * 128] * new[:, None]
            wts[:, bi * 128:(bi + 1) * 128] = wblk
        wts = (wts / sx).astype(np.float16)
        in_maps.append({"xin": xin, "wts": wts})

    # host residual: out lins [4096, 4100) (f-bin 1024), exact in fp32
    nzc = A[:L, RES_LO:L] != 0
    ri = int(np.nonzero(nzc.any(axis=1))[0].min())
    residual = A[ri:L, RES_LO:L].T @ X[ri:L]             # [4, R] fp32
    return in_maps, residual


def _gather_output(results, bias_img, residual):
    out_lin = np.zeros((L, R), np.float32)
    for c in range(NCORES):
        o8 = np.asarray(results[c]["out8"]).astype(np.float32) / SO
        for t, j in enumerate(CORE_TILES[c]):
            out_lin[j * 128:(j + 1) * 128] = o8[t * 128:(t + 1) * 128]
    out_lin[RES_LO:L] = residual
    out = out_lin.reshape(F, C, B, T).transpose(2, 1, 3, 0)
    out = np.ascontiguousarray(out) + bias_img[None, :, None, :]
    return out.astype(np.float32)


def _run_on_device(in_maps, loop_iters=1):
    from concourse.bass_utils import run_bass_kernel_spmd
    nc = _build_program(loop_iters)
    res = run_bass_kernel_spmd(nc, in_maps, list(range(NCORES)))
    return res.results


def kernel(x, pre_weight, pre_bias, post_weight, post_bias, mask, ola_window,
           f_idxes):
    x = np.asarray(x, np.float32)
    pre_weight = np.asarray(pre_weight, np.float32)
    pre_bias = np.asarray(pre_bias, np.float32)
    post_weight = np.asarray(post_weight, np.float32)
    post_bias = np.asarray(post_bias, np.float32)
    mask = np.asarray(mask, np.float32)
    ola_window = np.asarray(ola_window, np.float32)
    f_idxes = np.asarray(f_idxes)

    A, bias_img = _build_A(pre_weight, pre_bias, post_weight, post_bias,
                           mask, ola_window, f_idxes)
    in_maps, residual = _shard_inputs(x, A)
    results = _run_on_device(in_maps)
    return _gather_output(results, bias_img, residual)


# revision 7
# speedup vs baseline: 1.7942x; 1.7942x over previous
"""Trainium2 Bass kernel for nn_BandSplit (banded matmul, fp8 x, chain slices).

The reference pipeline (gather -> mask -> per-band linear -> linear -> mask ->
scatter_add -> OLA) is linear in x and collapses to ONE banded matrix multiply
in the interleaved linear space lin = f*4 + c:

    out_lin[l', r] = sum_l A[l, l'] * x_lin[l, r]        (r = b*T + t rows)

A is built on the host from the (small) weight inputs.  Each core owns 4
ADJACENT 128-wide output tiles (core c -> tiles 4c..4c+3); adjacent band
supports overlap strongly, so three of the tiles share a dense chain of five
128-row x-slices (host-chosen offsets, consecutive gaps <= 128; tile i is
covered by chain slices i-1, i, i+1 with duplicate/out-of-band rows zeroed in
the weights), while the narrowest ("odd") tile -- the group's first, or last
when the first is wider than 256 rows -- uses two private slices.  That is
NSL=7 x-slice loads and NDP=(2,3,3,3)=11 weight blocks per core, vs (8, 9) in
the per-band-slot layout and (6, 12) for a full 4-tile chain: one block fewer
keeps the PE stream (the bottleneck) at 11 x 2048 = 22.5K cycles ~ 9.4us.

Dtypes: x quantized host-side to fp8 E3M4 (scale SX folded into A), weights
fp16, PSUM fp32, ALL outputs stored fp8 E3M4 (x SO, divided out on the host).
Bias image and the 4 output lins above 4096 (f-bin 1024) are host-side.

Per-core steady-state budget: PE 22.5K cycles ~ 9.4us at 2.4GHz; DMA 3.24 MB
(x 1.83 + w 0.36 + out 1.05).  Loads ride the SP HWDGE queue; PSUM->SBUF
copies are split DVE (tiles 0-1) / ACT (tiles 2-3) since 16 on one engine
would bind; fp8 stores ride the gpsimd SWDGE (tiles 0-1) and ACT HWDGE
(tiles 2-3) queues, all separate from the load queue so consecutive bodies
pipeline.  TimelineSim steady-state body: 9372 ns.
"""

import numpy as np
import ml_dtypes

# ---- problem constants (hardcoded; harness supplies matching inputs) ----
B, C, T, F = 4, 4, 512, 1025
KB, WMAX = 256, 33
L = F * C                 # 4100 linear positions
R = B * T                 # 2048 rows (b, t)
NT_DEV = 32               # device out tiles (lin 0..4096); rest host residual
RES_LO = NT_DEV * 128     # 4096
NCORES = 8
CHUNK = 512               # PSUM bank (fp32) free-dim limit
NCH = R // CHUNK          # 4

# slot structure: tile 0 ("odd") reads private slices (5, 6); tiles 1-3
# form a dense chain over slices 0-4 (tile t reads slices (t-1, t, t+1))
NSL = 7                               # x slices per core (128 rows each)
NTPC = 4                              # out tiles per core
NDP = (2, 3, 3, 3)                    # weight blocks per tile
SMAP = ((5, 6), (0, 1, 2), (1, 2, 3), (2, 3, 4))
NBLK = sum(NDP)                       # 11 weight blocks per core

CORE_GROUPS = [tuple(range(4 * c, 4 * c + 4)) for c in range(NCORES)]
# per-core tile order (odd tile first) is data-dependent; _plan_slices
# stores it here for _gather_output
_TILE_ORDER = [None] * NCORES

SX_TARGET = 14.8          # fp8 e3m4 max normal is 15.5; leave clip margin
SO = 5.0                  # fp8 out scale (out absmax ~2.41 -> 12.1 < 15.5)

F8 = ml_dtypes.float8_e3m4

_prog_cache = {}


def _build_program(loop_iters=1, unroll=4):
    """loop_iters counts BODY executions; the hardware loop runs
    loop_iters/unroll iterations of `unroll` pipelined bodies (the revolving
    bufs=2 pools overlap consecutive bodies; the all-engine barrier sits on
    the loop back-edge only)."""
    import concourse.bacc as bacc
    import concourse.tile as tile
    import concourse.mybir as mybir

    if unroll and loop_iters % unroll:
        unroll = 1
    key = (loop_iters, unroll)
    if key in _prog_cache:
        return _prog_cache[key]

    f32 = mybir.dt.float32
    f16 = mybir.dt.float16
    f8 = mybir.dt.float8e3

    nc = bacc.Bacc("TRN2", target_bir_lowering=False, debug=False,
                   num_devices=NCORES)
    xin = nc.dram_tensor("xin", [128, NSL * R], f8, kind="ExternalInput").ap()
    wts = nc.dram_tensor("wts", [128, NBLK * 128], f16,
                         kind="ExternalInput").ap()
    out8 = nc.dram_tensor("out8", [NTPC * 128, R], f8,
                          kind="ExternalOutput").ap()

    with tile.TileContext(nc) as tc:
        with (
            tc.tile_pool(name="xp", bufs=2) as xp,
            tc.tile_pool(name="wp", bufs=2) as wp,
            tc.tile_pool(name="y8p", bufs=2) as y8p,
            tc.tile_pool(name="pp", bufs=8, space="PSUM") as pp,
        ):
            # x DRAM layout is (chunk, slice)-interleaved: col block
            # (ch*NSL + s)*CHUNK holds chunk ch of slice s, so each chunk is
            # one contiguous 384 KB load descriptor and compute can start
            # after w0 + chunk 0.  Matmuls run chunk-major so each chunk's
            # compute chases its load.
            def body(_iv=None):
                xt = xp.tile([128, NSL * R], f8, tag="x")
                wt0 = wp.tile([128, NDP[0] * 128], f16, tag="w0")
                wtr = wp.tile([128, (NBLK - NDP[0]) * 128], f16, tag="wr")
                xo = [NSL * CHUNK * ch for ch in range(NCH + 1)]
                nc.sync.dma_start(wt0[:], wts[:, :NDP[0] * 128])
                nc.sync.dma_start(xt[:, 0:xo[1]], xin[:, 0:xo[1]])
                nc.sync.dma_start(wtr[:], wts[:, NDP[0] * 128:])
                for ch in range(1, NCH):
                    nc.sync.dma_start(xt[:, xo[ch]:xo[ch + 1]],
                                      xin[:, xo[ch]:xo[ch + 1]])

                def wblk(t, b):
                    if t == 0:
                        return wt0[:, b * 128:(b + 1) * 128]
                    blk = (sum(NDP[:t]) - NDP[0] + b) * 128
                    return wtr[:, blk:blk + 128]

                y8s = [y8p.tile([128, R], f8, tag=f"y8_{t}",
                                name=f"y8_{t}") for t in range(NTPC)]

                for ch in range(NCH):
                    order = range(NTPC) if ch == 0 else (3, 0, 1, 2)
                    for t in order:
                        ps = pp.tile([128, CHUNK], f32, tag="ps")
                        for b in range(NDP[t]):
                            c0 = xo[ch] + SMAP[t][b] * CHUNK
                            nc.tensor.matmul(
                                ps[:], wblk(t, b), xt[:, c0:c0 + CHUNK],
                                start=(b == 0), stop=(b == NDP[t] - 1),
                            )
                        # PSUM->SBUF copies (with the SO scale folded in)
                        # split across DVE (tiles 0-1) and ACT (tiles 2-3):
                        # 16 copies on one engine would BE the bottleneck
                        # (~660ns each).  Stores: tiles 0-1 ride the gpsimd
                        # SWDGE queue (DVE cannot trigger HWDGE), tiles 2-3
                        # the ACT HWDGE queue right after their own copies;
                        # both are separate from the SP load queue so
                        # consecutive bodies overlap.
                        dst = y8s[t][:, ch * CHUNK:(ch + 1) * CHUNK]
                        if t < 2:
                            nc.vector.tensor_scalar_mul(dst, ps[:], SO)
                        else:
                            nc.scalar.mul(dst, ps[:], SO)
                        if ch == NCH - 1:
                            eng = nc.gpsimd if t < 2 else nc.scalar
                            eng.dma_start(
                                out8[t * 128:(t + 1) * 128, :], y8s[t][:])

            if loop_iters == 1:
                body()
            elif unroll == 0:
                # straight-line replay (no For_i): TimelineSim cannot follow
                # reg-mode branches, so simulation uses this variant
                for _u in range(loop_iters):
                    body()
            else:
                with tc.For_i(0, loop_iters // unroll, 1) as _i:
                    for _u in range(unroll):
                        body(_i)

    nc.compile()
    _prog_cache[key] = nc
    return nc


def _build_A(pre_weight, pre_bias, post_weight, post_bias, mask, ola_window,
             f_idxes):
    """Host: banded operator A[in_lin, out_lin] and the bias image (C, F)."""
    fi = f_idxes.reshape(KB, WMAX).astype(np.int64)
    mk = mask.reshape(KB, WMAX).astype(np.float32)
    ola = ola_window.astype(np.float32)

    mrow = np.repeat(mk, C, axis=1)                     # (KB, WMAX*C)
    inv_ola = np.where(ola != 0, 1.0 / ola, 0.0)
    ola_cols = inv_ola[fi]                              # (KB, WMAX)
    mcol = np.repeat(mk * ola_cols, C, axis=1)          # (KB, WMAX*C)

    w1 = pre_weight * mrow[:, :, None]                  # (KB, D, 128)
    w2 = post_weight * mcol[:, None, :]                 # (KB, 128, D)
    Mk = np.matmul(w1, w2)                              # (KB, D, D) fp32

    LPAD = ((L + 127) // 128) * 128
    A = np.zeros((LPAD, LPAD), np.float32)
    lin = (fi[:, :, None] * C + np.arange(C)[None, None, :]).reshape(KB, -1)
    for k in range(KB):
        idx = lin[k]
        A[np.ix_(idx, idx)] += Mk[k]

    by = (np.einsum('ko,koj->kj', pre_bias, post_weight) + post_bias)
    by = by * mcol
    bias_img = np.zeros((C, F), np.float32)
    np.add.at(bias_img,
              (np.tile(np.arange(C), (KB, WMAX, 1)).reshape(KB, -1),
               np.repeat(fi, C, axis=1)),
              by)
    return A, bias_img


def _plan_slices(A):
    """Per-core slice offsets + per-block (tile, offset, new-row mask).

    Core c owns tiles 4c..4c+3.  The "odd" tile (first tile, or last if the
    first needs >2 slices) gets private slices 5-6; the remaining 3 adjacent
    tiles chain over slices 0-4 (tile i covered by [offs[i-1], offs[i+1]+128)
    with non-decreasing offsets, consecutive gaps <= 128).
    """
    sup = []
    nzc = A[:L, :RES_LO] != 0
    for j in range(NT_DEV):
        rows = np.nonzero(nzc[:, 128 * j:128 * (j + 1)].any(axis=1))[0]
        sup.append((int(rows.min()), int(rows.max())))

    slice_offs, blocks = [], []
    for c in range(NCORES):
        grp = CORE_GROUPS[c]
        w_first = sup[grp[0]][1] - sup[grp[0]][0] + 1
        if w_first <= 256:
            tiles = [grp[0], grp[1], grp[2], grp[3]]
        else:
            tiles = [grp[3], grp[0], grp[1], grp[2]]
        assert sup[tiles[0]][1] - sup[tiles[0]][0] + 1 <= 256, (c, tiles)
        _TILE_ORDER[c] = tuple(tiles)
        los = [sup[j][0] for j in tiles]
        his = [sup[j][1] for j in tiles]
        # chain slices 0-4 cover tiles 1-3
        offs = []
        for i in range(5):
            o = L - 128
            if i < 3:
                o = min(o, los[i + 1])
            if i > 0:
                o = min(o, offs[i - 1] + 128)
            offs.append(max(0, o))
        for i in range(3):
            assert offs[i] <= los[i + 1] and offs[i + 2] + 128 > his[i + 1], \
                (c, i, offs, los, his)
        # odd tile private slices 5-6
        o5 = max(0, min(los[0], L - 128))
        o6 = max(0, min(o5 + 128, L - 128))
        assert o6 + 128 > his[0], (c, offs, los, his)
        offs += [o5, o6]

        blks = []
        for t, j in enumerate(tiles):
            covered = np.zeros(L + 128, bool)
            for b in range(NDP[t]):
                o = offs[SMAP[t][b]]
                new = ~covered[o:o + 128]
                blks.append((j, o, new.copy()))
                covered[o:o + 128] = True
            assert covered[sup[j][0]:sup[j][1] + 1].all()
        slice_offs.append(offs)
        blocks.append(blks)
    return slice_offs, blocks


def _shard_inputs(x, A):
    """Per-core in_maps plus host-side residual rows (lin 4096..4099)."""
    X = np.ascontiguousarray(
        np.asarray(x, np.float32).transpose(3, 1, 0, 2).reshape(L, R))
    sx = SX_TARGET / max(float(np.abs(X).max()), 1e-30)
    Xq = np.clip(X * sx, -15.5, 15.5).astype(F8)

    slice_offs, blocks = _plan_slices(A)
    in_maps = []
    for c in range(NCORES):
        # (chunk, slice)-interleaved column layout matching _build_program
        xin = np.zeros((128, NSL * R), F8)
        for s, o in enumerate(slice_offs[c]):
            for ch in range(NCH):
                d0 = (ch * NSL + s) * CHUNK
                xin[:, d0:d0 + CHUNK] = Xq[o:o + 128,
                                           ch * CHUNK:(ch + 1) * CHUNK]
        wts = np.zeros((128, NBLK * 128), np.float32)
        for bi, (j, o, new) in enumerate(blocks[c]):
            wblk = A[o:o + 128, j * 128:(j + 1) * 128] * new[:, None]
            wts[:, bi * 128:(bi + 1) * 128] = wblk
        wts = (wts / sx).astype(np.float16)
        in_maps.append({"xin": xin, "wts": wts})

    # host residual: out lins [4096, 4100) (f-bin 1024), exact in fp32
    nzc = A[:L, RES_LO:L] != 0
    ri = int(np.nonzero(nzc.any(axis=1))[0].min())
    residual = A[ri:L, RES_LO:L].T @ X[ri:L]             # [4, R] fp32
    return in_maps, residual


def _gather_output(results, bias_img, residual):
    out_lin = np.zeros((L, R), np.float32)
    for c in range(NCORES):
        o8 = np.asarray(results[c]["out8"]).astype(np.float32) / SO
        for t, j in enumerate(_TILE_ORDER[c]):
            out_lin[j * 128:(j + 1) * 128] = o8[t * 128:(t + 1) * 128]
    out_lin[RES_LO:L] = residual
    out = out_lin.reshape(F, C, B, T).transpose(2, 1, 3, 0)
    out = np.ascontiguousarray(out) + bias_img[None, :, None, :]
    return out.astype(np.float32)


def _run_on_device(in_maps, loop_iters=1):
    from concourse.bass_utils import run_bass_kernel_spmd
    nc = _build_program(loop_iters)
    res = run_bass_kernel_spmd(nc, in_maps, list(range(NCORES)))
    return res.results


def kernel(x, pre_weight, pre_bias, post_weight, post_bias, mask, ola_window,
           f_idxes):
    x = np.asarray(x, np.float32)
    pre_weight = np.asarray(pre_weight, np.float32)
    pre_bias = np.asarray(pre_bias, np.float32)
    post_weight = np.asarray(post_weight, np.float32)
    post_bias = np.asarray(post_bias, np.float32)
    mask = np.asarray(mask, np.float32)
    ola_window = np.asarray(ola_window, np.float32)
    f_idxes = np.asarray(f_idxes)

    A, bias_img = _build_A(pre_weight, pre_bias, post_weight, post_bias,
                           mask, ola_window, f_idxes)
    in_maps, residual = _shard_inputs(x, A)
    results = _run_on_device(in_maps)
    return _gather_output(results, bias_img, residual)
